# revision 1
# baseline (speedup 1.0000x reference)
"""Trainium2 Bass kernel for a 3x3 stride-1 pad-1 conv2d (LoopConv2d).

Problem: x[16, 64, 112, 112] (f32), w[128, 64, 3, 3], b[128]
         -> out[16, 128, 112, 112]  (out = conv2d(x, w) + b)

Strategy
--------
Data-parallel over batch: 16 images / 8 cores = 2 images per core.

Per core, direct convolution as PE matmuls accumulating in PSUM:
  out[co, pix] += sum_ci w[co, ci, dy, dx] * x[ci, pix + (dy, dx)]

The contraction dim (C_IN = 64) only half-fills the 128-row PE array, so
input rows are parity-packed in SBUF: partition k < 64 holds channel k of
EVEN image rows, partition 64+k holds channel k of ODD rows, with row pair
(2s, 2s+1) sharing column-slot s.  A K=128 matmul over slot s then
contracts TWO vertical taps (dy, dy+1) at once.  Per output-row parity
group, 2 of the 3 vertical taps merge into one K=128 matmul; the third is
a K=64 matmul on one partition half (row-tiled, so the even-group and
odd-group leftovers can overlap on the PE).

Rows are padded to WP=114 (1 zero col each side) and the row-index space
is shifted by +2 (idx = row + 2, idx 0..115 -> 58 slots) so the dy=-1 tap
of output row 0 and dy=+1 of row 111 read zero slots - no edge branches.

Matmul free dim spans G=4 output rows of one parity = 456 columns <= 512
(one PSUM bank); the 2 junk columns per row (conv windows straddling row
ends) are simply not copied out.

The K=64 leftover matmuls are emitted as same-weight runs on disjoint
PE row halves (partitions 64:128 then 0:64), which the PE executes
concurrently (measured ~172ns/matmul < the 190ns serial floor).  Both
images' x loads are issued up front on the SP HWDGE ring; output stores
ride the ACT HWDGE ring, so store-waits never queue ahead of loads.

dtype: bf16 x/w/out with f32 PSUM accumulation (1 cycle/row on the PE,
half the HBM traffic of f32, fast weight loads).  Rel l2 err vs the f32
jax reference: 2.7e-3 (threshold 2e-2).

Weights / bias / x are pre-arranged on host (numpy) into the exact SBUF
layouts so every DMA is a large contiguous copy.
"""

import numpy as np
import jax

import concourse.bass as bass
import concourse.tile as tile
from concourse import bacc, mybir
from concourse import bass2jax
from jax.experimental.shard_map import shard_map
from jax.sharding import Mesh, PartitionSpec

B, C_IN, H, W = 16, 64, 112, 112
C_OUT, KH, KW = 128, 3, 3
N_CORES = 8
WP = W + 2  # padded row width

F32 = mybir.dt.float32


def _prep_x(x: np.ndarray, h: int) -> np.ndarray:
    """[b, C_IN, h, W] f32 -> [b, 128, HP*WP + 2] parity-packed padded rows.

    idx = row + 2; slot s holds idx rows (2s, 2s+1) on partition halves
    (lower, upper); idx rows 0,1 (= real -2,-1) and h+2..h+3 are zeros.
    """
    nb = x.shape[0]
    hp = (h + 4) // 2  # slots
    xpad = np.zeros((nb, C_IN, 2 * hp, WP), dtype=np.float32)
    xpad[:, :, 2 : h + 2, 1 : W + 1] = x
    out = np.zeros((nb, 2 * C_IN, hp * WP + 2), dtype=np.float32)
    out[:, :C_IN, : hp * WP] = xpad[:, :, 0::2, :].reshape(nb, C_IN, hp * WP)
    out[:, C_IN:, : hp * WP] = xpad[:, :, 1::2, :].reshape(nb, C_IN, hp * WP)
    return out


def _prep_w(w: np.ndarray) -> np.ndarray:
    """[C_OUT, C_IN, 3, 3] -> [128, 9, C_OUT] stationary-operand configs.

    cfg 0..2  (even-parity K=128, dx=cfg):   lower w[:,:,1,dx], upper w[:,:,2,dx]
    cfg 3..5  (odd-parity  K=128, dx=cfg-3): lower w[:,:,0,dx], upper w[:,:,1,dx]
    cfg 6..8  (K=64 leftovers, dx=cfg-6):    lower w[:,:,2,dx] (odd dy=+1),
                                             upper w[:,:,0,dx] (even dy=-1)
    """
    wt = w.transpose(1, 2, 3, 0).astype(np.float32)  # [ci, kh, kw, co]
    out = np.empty((2 * C_IN, 9, C_OUT), dtype=np.float32)
    for dx in range(3):
        out[:C_IN, 0 + dx] = wt[:, 1, dx]
        out[C_IN:, 0 + dx] = wt[:, 2, dx]
        out[:C_IN, 3 + dx] = wt[:, 0, dx]
        out[C_IN:, 3 + dx] = wt[:, 1, dx]
        out[:C_IN, 6 + dx] = wt[:, 2, dx]
        out[C_IN:, 6 + dx] = wt[:, 0, dx]
    return out


def build(
    b_sh: int,
    h: int,
    mm_dt=mybir.dt.bfloat16,
    w_dt=None,
    out_dt=mybir.dt.bfloat16,
    repeat: int = 1,
    loop: int = 0,
    bs: int = 4,
    prefetch: bool = True,
    store_eng: str = "scalar",
    chunks: str = "v1",
    keep_warm: bool = False,
):
    """Build the per-core Bass program. h must be divisible by 8.

    mm_dt: dtype of the moving operand (x); w_dt: dtype of the stationary
    operand (weights), defaults to mm_dt.
    repeat > 1 re-runs the whole conv back to back (python-unrolled).
    loop > 0 wraps the conv in a hardware For_i loop running it `loop`
    times (for timing; output is just overwritten each iteration).
    """
    assert h % 8 == 0
    if w_dt is None:
        w_dt = mm_dt
    nt = h // 8  # pair-units (8 output rows each)
    hp = (h + 4) // 2  # slots
    xcols = hp * WP + 2
    nfree = 4 * WP  # matmul free dim (456)

    nc = bacc.Bacc(
        "TRN2", target_bir_lowering=False, debug=False, num_devices=N_CORES
    )
    x_d = nc.dram_tensor("xprep", [b_sh, 128, xcols], mm_dt, kind="ExternalInput").ap()
    w_d = nc.dram_tensor("wprep", [128, 9, C_OUT], w_dt, kind="ExternalInput").ap()
    b_d = nc.dram_tensor("bias", [C_OUT, 1], F32, kind="ExternalInput").ap()
    o_d = nc.dram_tensor("out", [b_sh, C_OUT, h, W], out_dt, kind="ExternalOutput").ap()

    from contextlib import ExitStack, nullcontext

    with tile.TileContext(nc) as tc:
        with (
            tc.tile_pool(name="wpool", bufs=1) as wpool,
            tc.tile_pool(name="xpool", bufs=2) as xpool,
            tc.tile_pool(name="stage", bufs=6) as stage,
            tc.tile_pool(name="psum", bufs=8, space="PSUM") as pspool,
        ):
            wt = wpool.tile([128, 9, C_OUT], w_dt)
            bt = wpool.tile([C_OUT, 1], F32)
            nc.sync.dma_start(wt[:], w_d[:])
            nc.sync.dma_start(bt[:], b_d[:])

            def emit_conv(warm: bool = False):
                if warm:
                    # the x-load head idles the PE for ~3.6us, just past the
                    # HAM MID window (~3.4us) that re-throttles it to 1.2GHz.
                    # ~0.9us of junk matmuls on the resident weight tile keep
                    # the idle gap under the window; the first real
                    # start=True matmul clears the bank anyway.
                    wv = wt[:].rearrange("p a b -> p (a b)")  # [128, 1152]
                    pw = pspool.tile([C_OUT, nfree], F32, tag="ps")
                    for j in range(4):
                        c0 = 456 * (j % 2)
                        nc.tensor.matmul(
                            pw[:, :],
                            wt[:, 0, :],
                            wv[:, c0 : c0 + nfree],
                            start=(j == 0),
                            stop=(j == 3),
                            skip_group_check=True,
                        )
                # chunked load: first matmuls start after ~1/4 of x lands.
                # v2 aligns chunk 1 with the first cfg-run's read range
                # (slots 1-16 for bs=4) so the head waits on one chunk only.
                if chunks == "v2":
                    bounds = [0, 17 * WP, 31 * WP, 45 * WP, xcols]
                else:
                    bounds = [0, 15 * WP, 30 * WP, 45 * WP, xcols]
                xts = {}
                if prefetch:
                    # issue every image's loads up front so no load trigger
                    # queues behind a store-wait on the DMA sequencer
                    for b in range(b_sh):
                        xt = xpool.tile([128, xcols], mm_dt, tag="x")
                        for a, c in zip(bounds[:-1], bounds[1:]):
                            nc.sync.dma_start(xt[:, a:c], x_d[b, :, a:c])
                        xts[b] = xt
                for b in range(b_sh):
                    if prefetch:
                        xt = xts[b]
                    else:
                        xt = xpool.tile([128, xcols], mm_dt, tag="x")
                        for a, c in zip(bounds[:-1], bounds[1:]):
                            nc.sync.dma_start(xt[:, a:c], x_d[b, :, a:c])

                    # batches of `bs` pair-units (2*bs PSUM banks; spare
                    # banks let the next batch's start-matmuls proceed
                    # while the previous batch evacuates), cfg-outer so
                    # each stationary operand streams bs rhs tiles in a row
                    for tb in range(0, nt, bs):
                        ts = list(range(tb, min(tb + bs, nt)))
                        pse, pso = {}, {}
                        for t in ts:
                            ps_te = pspool.tile([C_OUT, nfree], F32, tag="ps")
                            pse[t] = ps_te
                            ps_to = pspool.tile([C_OUT, nfree], F32, tag="ps")
                            pso[t] = ps_to
                        # even groups (out rows 8t, 8t+2, 8t+4, 8t+6)
                        for dx in range(3):
                            for t in ts:
                                c0 = (4 * t + 1) * WP + dx
                                nc.tensor.matmul(
                                    pse[t][:, :],
                                    wt[:, dx, :],
                                    xt[:, c0 : c0 + nfree],
                                    start=(dx == 0),
                                    stop=False,
                                )
                        # odd groups (out rows 8t+1 .. 8t+7)
                        for dx in range(3):
                            for t in ts:
                                c0 = (4 * t + 1) * WP + dx
                                nc.tensor.matmul(
                                    pso[t][:, :],
                                    wt[:, 3 + dx, :],
                                    xt[:, c0 : c0 + nfree],
                                    start=(dx == 0),
                                    stop=False,
                                )
                        # K=64 leftovers: same-weight runs of len(ts) (one
                        # stationary load each) on disjoint row halves, so
                        # the even-run and odd-run overlap on the PE.
                        for dx in range(3):
                            for t in ts:
                                ce = (4 * t) * WP + dx
                                nc.tensor.matmul(
                                    pse[t][:, :],
                                    wt[64:128, 6 + dx, :],
                                    xt[64:128, ce : ce + nfree],
                                    start=False,
                                    stop=(dx == 2),
                                )
                            for t in ts:
                                co = (4 * t + 2) * WP + dx
                                nc.tensor.matmul(
                                    pso[t][:, :],
                                    wt[0:64, 6 + dx, :],
                                    xt[0:64, co : co + nfree],
                                    start=False,
                                    stop=(dx == 2),
                                )
                        # bias + evacuate PSUM -> staging [co, 4, 2, W]
                        for t in ts:
                            st = stage.tile([C_OUT, 4, 2, W], out_dt, tag="st")
                            ev = pse[t][:].rearrange("p (g w) -> p g w", w=WP)[
                                :, :, 0:W
                            ]
                            ov = pso[t][:].rearrange("p (g w) -> p g w", w=WP)[
                                :, :, 0:W
                            ]
                            nc.scalar.activation(
                                st[:, :, 0, :],
                                ev,
                                mybir.ActivationFunctionType.Identity,
                                bias=bt[:, 0:1],
                            )
                            nc.vector.tensor_scalar_add(
                                st[:, :, 1, :], ov, bt[:, 0:1]
                            )
                            # stores ride the ACT HWDGE ring so their
                            # evac-waits never block x loads on the SP ring
                            store_ns = getattr(nc, store_eng)
                            store_ns.dma_start(
                                o_d[b, :, 8 * t : 8 * t + 8, :], st[:]
                            )

            if loop > 0:
                with tc.For_i(0, loop, 1, hint_engines=(mybir.EngineType.PE,)):
                    emit_conv(warm=keep_warm)
            else:
                for _rep in range(repeat):
                    emit_conv(warm=keep_warm)

    nc.compile()
    return nc


class Runner:
    """Persistent jitted shard_map executor for a compiled Bass program.

    Mirrors concourse.bass2jax.run_bass_via_pjrt's multi-core path but
    caches the jitted function so repeated calls skip re-tracing.
    """

    def __init__(self, nc, n_cores: int = N_CORES):
        bass2jax.install_neuronx_cc_hook()
        assert nc.dbg_addr is None
        self.nc = nc
        self.n_cores = n_cores
        partition_name = (
            nc.partition_id_tensor.name if nc.partition_id_tensor else None
        )
        in_names: list[str] = []
        out_names: list[str] = []
        out_avals: list[jax.core.ShapedArray] = []
        for alloc in nc.m.functions[0].allocations:
            if not isinstance(alloc, mybir.MemoryLocationSet):
                continue
            name = alloc.memorylocations[0].name
            if alloc.kind == "ExternalInput":
                if name != partition_name:
                    in_names.append(name)
            elif alloc.kind == "ExternalOutput":
                out_names.append(name)
                out_avals.append(
                    jax.core.ShapedArray(
                        tuple(alloc.tensor_shape), mybir.dt.np(alloc.dtype)
                    )
                )
        self.in_names = in_names
        self.out_names = out_names
        self.out_avals = out_avals
        self.in_dtypes = {}
        for alloc in nc.m.functions[0].allocations:
            if (
                isinstance(alloc, mybir.MemoryLocationSet)
                and alloc.kind == "ExternalInput"
            ):
                self.in_dtypes[alloc.memorylocations[0].name] = mybir.dt.np(
                    alloc.dtype
                )
        n_params = len(in_names)
        n_outs = len(out_names)
        all_names = list(in_names) + list(out_names)
        if partition_name is not None:
            all_names.append(partition_name)
        all_names = tuple(all_names)

        def _body(*args):
            operands = list(args)
            if partition_name is not None:
                operands.append(bass2jax.partition_id_tensor())
            outs = bass2jax._bass_exec_p.bind(
                *operands,
                out_avals=tuple(out_avals),
                in_names=all_names,
                out_names=tuple(out_names),
                lowering_input_output_aliases=(),
                sim_require_finite=True,
                sim_require_nnan=True,
                nc=nc,
            )
            return tuple(outs)

        devices = jax.devices()[:n_cores]
        assert len(devices) == n_cores
        self.mesh = Mesh(np.asarray(devices), ("core",))
        in_specs = (PartitionSpec("core"),) * (n_params + n_outs)
        out_specs = (PartitionSpec("core"),) * n_outs
        donate = tuple(range(n_params, n_params + n_outs))
        self.fn = jax.jit(
            shard_map(
                _body,
                mesh=self.mesh,
                in_specs=in_specs,
                out_specs=out_specs,
                check_rep=False,
            ),
            donate_argnums=donate,
            keep_unused=True,
        )

    def concat_inputs(self, in_maps):
        return [
            np.concatenate(
                [
                    np.asarray(m[name]).astype(self.in_dtypes[name], copy=False)
                    for m in in_maps
                ],
                axis=0,
            )
            for name in self.in_names
        ]

    def zero_outs(self):
        return [
            np.zeros((self.n_cores * a.shape[0], *a.shape[1:]), a.dtype)
            for a in self.out_avals
        ]

    def call_raw(self, concat_in, zeros):
        """concat_in/zeros may be np or device arrays. Returns jax arrays."""
        return self.fn(*concat_in, *zeros)

    def __call__(self, in_maps):
        outs = self.call_raw(self.concat_inputs(in_maps), self.zero_outs())
        outs = [np.asarray(o) for o in outs]
        return [
            {
                name: outs[i].reshape(self.n_cores, *self.out_avals[i].shape)[c]
                for i, name in enumerate(self.out_names)
            }
            for c in range(self.n_cores)
        ]


_CACHE: dict = {}


def get_runner(repeat: int = 1, loop: int = 0, **kw) -> Runner:
    key = ("full", repeat, loop, tuple(sorted(kw.items())))
    if key not in _CACHE:
        nc = build(B // N_CORES, H, repeat=repeat, loop=loop, **kw)
        _CACHE[key] = Runner(nc)
    return _CACHE[key]


def make_in_maps(x, w, b):
    b_sh = B // N_CORES
    wp = _prep_w(np.asarray(w))
    bp = np.asarray(b).astype(np.float32).reshape(C_OUT, 1)
    xp = _prep_x(np.asarray(x, dtype=np.float32), H)
    return [
        {"xprep": xp[i * b_sh : (i + 1) * b_sh], "wprep": wp, "bias": bp}
        for i in range(N_CORES)
    ]


def kernel(x, w, b):
    runner = get_runner()
    res = runner(make_in_maps(x, w, b))
    out = np.concatenate([r["out"] for r in res], axis=0)
    return np.asarray(out, dtype=np.float32)



# revision 32
# speedup vs baseline: 1.0088x; 1.0088x over previous
"""Trainium2 Bass kernel for a 3x3 stride-1 pad-1 conv2d (LoopConv2d).

Problem: x[16, 64, 112, 112] (f32), w[128, 64, 3, 3], b[128]
         -> out[16, 128, 112, 112]  (out = conv2d(x, w) + b)

Strategy
--------
Data-parallel over batch: 16 images / 8 cores = 2 images per core.

Per core, direct convolution as PE matmuls accumulating in PSUM:
  out[co, pix] += sum_ci w[co, ci, dy, dx] * x[ci, pix + (dy, dx)]

The contraction dim (C_IN = 64) only half-fills the 128-row PE array, so
input rows are parity-packed in SBUF: partition k < 64 holds channel k of
EVEN image rows, partition 64+k holds channel k of ODD rows, with row pair
(2s, 2s+1) sharing column-slot s.  A K=128 matmul over slot s then
contracts TWO vertical taps (dy, dy+1) at once.  Per output-row parity
group, 2 of the 3 vertical taps merge into one K=128 matmul; the third is
a K=64 matmul on one partition half.

The K=64 leftovers for the even group live on partitions 64:128 and for
the odd group on partitions 0:64; they are emitted INTERLEAVED (u, l, u,
l, ...) so consecutive matmuls target disjoint PE row-groups and may
co-execute on the array (tile_position row tiling).

Rows are padded to WP=114 (1 zero col each side) and the row-index space
is shifted by +2 (idx = row + 2, idx 0..115 -> 58 slots) so the dy=-1 tap
of output row 0 and dy=+1 of row 111 read zero slots - no edge branches.

Matmul free dim spans G=4 output rows of one parity = 456 columns <= 512
(one PSUM bank); the 2 junk columns per row (conv windows straddling row
ends) are simply not copied out.

Pair-units are processed in batches sized by `bplan`: single-shot builds
use small head batches so compute starts after a tiny first x chunk; x is
DMA'd in per-batch chunks whose bounds exactly cover each batch's read
range, so no matmul ever waits on more data than it needs.

tile_legalize pairs every matmul with its own Ldweights; a post-legalize
pass (_dedup_ldweights) drops reloads of weights already resident in the
targeted PE row-group, so each same-weight run loads its stationary
operand once (336 -> 144 Ldweights per conv; numerics verified on HW).

For timing builds (loop > 0) the conv body runs under
For_i(staggered_reset=True) - no all-engine barrier + drain per iteration
- and each body runs `unroll` convs with software-pipelined x double
buffering: the loads for conv k+1 are issued while conv k computes, so
the first matmul after a loop back-edge never waits on HBM.  Measured
non-PE overhead (DMA + evac + stores + stage barriers) is ~9us/conv over
a pure-matmul stream; the wall is the PE stream rate itself (the device
streams N=456 bf16 matmuls at ~217-255 ns instead of the warm-clock 190,
consistent with the documented HAM oscillation on PSUM bank cycling).

dtype: bf16 x/w/out with f32 PSUM accumulation.  Rel l2 err vs the f32
jax reference: 2.7e-3 (threshold 2e-2).
"""

import numpy as np
import jax

import concourse.bass as bass
import concourse.tile as tile
from concourse import bacc, mybir
from concourse import bass2jax
from jax.experimental.shard_map import shard_map
from jax.sharding import Mesh, PartitionSpec

B, C_IN, H, W = 16, 64, 112, 112
C_OUT, KH, KW = 128, 3, 3
N_CORES = 8
WP = W + 2  # padded row width

F32 = mybir.dt.float32


def _prep_x(x: np.ndarray, h: int) -> np.ndarray:
    """[b, C_IN, h, W] f32 -> [b, 128, HP*WP + 2] parity-packed padded rows.

    idx = row + 2; slot s holds idx rows (2s, 2s+1) on partition halves
    (lower, upper); idx rows 0,1 (= real -2,-1) and h+2..h+3 are zeros.
    """
    nb = x.shape[0]
    hp = (h + 4) // 2  # slots
    xpad = np.zeros((nb, C_IN, 2 * hp, WP), dtype=np.float32)
    xpad[:, :, 2 : h + 2, 1 : W + 1] = x
    out = np.zeros((nb, 2 * C_IN, hp * WP + 2), dtype=np.float32)
    out[:, :C_IN, : hp * WP] = xpad[:, :, 0::2, :].reshape(nb, C_IN, hp * WP)
    out[:, C_IN:, : hp * WP] = xpad[:, :, 1::2, :].reshape(nb, C_IN, hp * WP)
    return out


def _prep_w(w: np.ndarray) -> np.ndarray:
    """[C_OUT, C_IN, 3, 3] -> [128, 9, C_OUT] stationary-operand configs.

    cfg 0..2  (even-parity K=128, dx=cfg):   lower w[:,:,1,dx], upper w[:,:,2,dx]
    cfg 3..5  (odd-parity  K=128, dx=cfg-3): lower w[:,:,0,dx], upper w[:,:,1,dx]
    cfg 6..8  (K=64 leftovers, dx=cfg-6):    lower w[:,:,2,dx] (odd dy=+1),
                                             upper w[:,:,0,dx] (even dy=-1)
    """
    wt = w.transpose(1, 2, 3, 0).astype(np.float32)  # [ci, kh, kw, co]
    out = np.empty((2 * C_IN, 9, C_OUT), dtype=np.float32)
    for dx in range(3):
        out[:C_IN, 0 + dx] = wt[:, 1, dx]
        out[C_IN:, 0 + dx] = wt[:, 2, dx]
        out[:C_IN, 3 + dx] = wt[:, 0, dx]
        out[C_IN:, 3 + dx] = wt[:, 1, dx]
        out[:C_IN, 6 + dx] = wt[:, 2, dx]
        out[C_IN:, 6 + dx] = wt[:, 0, dx]
    return out


def _dedup_ldweights(ordered: dict) -> int:
    """Remove InstLdweights that reload weights already resident in the PE
    array.  The array's 32-row groups hold weights independently (row
    tiling), so state is tracked per partition range; a load clobbers only
    overlapping ranges.  Runs on tile_legalize output, before sem
    assignment, so no sync_info needs moving."""
    removed = 0
    for bb, insts in ordered.items():
        state: dict = {}
        keep = []
        for inst in insts:
            if inst.engine != mybir.EngineType.PE:
                keep.append(inst)
                continue
            if isinstance(inst, mybir.InstLdweights):
                ap = inst.ins[0]
                bap = ap.bass_ap
                if bap is None:
                    state.clear()
                    keep.append(inst)
                    continue
                try:
                    p0 = bap.base_partition()
                    pn = bap.partition_size()
                    wk = (bap.name, bap.offset, str(bap.ap), str(ap.dtype))
                except Exception:
                    state.clear()
                    keep.append(inst)
                    continue
                rk = (p0, pn)
                if state.get(rk) == wk:
                    removed += 1
                    continue  # drop: identical weights already loaded
                # clobber overlapping row ranges
                for (q0, qn) in list(state):
                    if q0 < p0 + pn and p0 < q0 + qn and (q0, qn) != rk:
                        del state[(q0, qn)]
                state[rk] = wk
                keep.append(inst)
            elif isinstance(inst, mybir.InstMatmult):
                keep.append(inst)
            elif isinstance(inst, (mybir.InstEventSemaphore, mybir.InstNoOp)):
                keep.append(inst)
            else:
                # branches / drains / anything unusual: forget state
                state.clear()
                keep.append(inst)
        insts[:] = keep
    return removed


def _batch_plan(nt: int, bplan) -> list[list[int]]:
    """Resolve a batch plan into a list of pair-unit index lists."""
    if bplan == "v1":
        sizes = []
        r = nt
        while r > 0:
            sizes.append(min(4, r))
            r -= sizes[-1]
    elif bplan == "fine":
        sizes = [1, 1, 2] + [4] * ((nt - 4) // 4)
        rem = nt - sum(sizes)
        if rem:
            sizes.append(rem)
    elif isinstance(bplan, (tuple, list)):
        sizes = list(bplan)
        assert sum(sizes) == nt
    else:
        raise ValueError(f"bad bplan {bplan!r}")
    out = []
    t0 = 0
    for s in sizes:
        out.append(list(range(t0, t0 + s)))
        t0 += s
    return out


def build(
    b_sh: int,
    h: int,
    mm_dt=mybir.dt.bfloat16,
    w_dt=None,
    out_dt=mybir.dt.bfloat16,
    repeat: int = 1,
    loop: int = 0,
    bplan="fine",
    prefetch: bool = True,
    store_eng: str = "scalar",
    interleave: bool = True,
    staggered: bool = True,
    keep_warm: bool = False,
    dedup: bool = True,
    pe2x: bool = False,
    unroll: int = 2,
    mmonly: bool = False,
):
    """Build the per-core Bass program. h must be divisible by 8.

    mm_dt: dtype of the moving operand (x); w_dt: dtype of the stationary
    operand (weights), defaults to mm_dt.
    repeat > 1 re-runs the whole conv back to back (python-unrolled).
    loop > 0 wraps the conv in a hardware For_i loop running it `loop`
    times (for timing; output is just overwritten each iteration).
    store_eng: 'scalar' | 'vector' | 'alt' (alternate per pair-unit).
    interleave: emit the K=64 leftover matmuls u/l interleaved so they
    run concurrently on disjoint PE row halves.
    staggered: use For_i(staggered_reset=True) for loop builds.
    """
    assert h % 8 == 0
    if w_dt is None:
        w_dt = mm_dt
    if loop > 0 and bplan == "fine":
        # pipelined loop pre-loads x a full conv ahead; the small head
        # batches only help single-shot latency, while fewer/larger
        # batches mean fewer weight switches
        bplan = "v1"
    nt = h // 8  # pair-units (8 output rows each)
    hp = (h + 4) // 2  # slots
    xcols = hp * WP + 2
    nfree = 4 * WP  # matmul free dim (456)

    batches = _batch_plan(nt, bplan)
    max_bs = max(len(ts) for ts in batches)

    # per-batch exact max read column (the odd-group leftover reads
    # (4t+2)*WP + dx + nfree); chunk bounds = cumulative batch needs
    bounds = [0]
    for ts in batches:
        tmax = ts[-1]
        need = min((4 * tmax + 2) * WP + 2 + nfree, xcols)
        if need > bounds[-1]:
            bounds.append(need)
    if bounds[-1] < xcols:
        bounds.append(xcols)

    nc = bacc.Bacc(
        "TRN2", target_bir_lowering=False, debug=False, num_devices=N_CORES
    )
    x_d = nc.dram_tensor("xprep", [b_sh, 128, xcols], mm_dt, kind="ExternalInput").ap()
    w_d = nc.dram_tensor("wprep", [128, 9, C_OUT], w_dt, kind="ExternalInput").ap()
    b_d = nc.dram_tensor("bias", [C_OUT, 1], F32, kind="ExternalInput").ap()
    o_d = nc.dram_tensor("out", [b_sh, C_OUT, h, W], out_dt, kind="ExternalOutput").ap()

    if dedup:
        # intercept tile_legalize output (pre sem-assignment) to drop
        # redundant weight reloads; restored in the finally below
        _orig_legalize = tile.tile_legalize

        def _patched(ordered, nc_):
            out = _orig_legalize(ordered, nc_)
            n = _dedup_ldweights(out)
            _patched.removed = n
            return out

        _patched.removed = 0
        tile.tile_legalize = _patched

    try:
        _build_body(
            nc, b_sh, loop, repeat, mm_dt, w_dt, out_dt, prefetch, store_eng,
            interleave, staggered, keep_warm, batches, bounds, xcols, nfree,
            x_d, w_d, b_d, o_d, pe2x, unroll, mmonly,
        )
    finally:
        if dedup:
            tile.tile_legalize = _orig_legalize

    nc.compile()
    return nc


def _build_body(
    nc, b_sh, loop, repeat, mm_dt, w_dt, out_dt, prefetch, store_eng,
    interleave, staggered, keep_warm, batches, bounds, xcols, nfree,
    x_d, w_d, b_d, o_d, pe2x=False, unroll=2, mmonly=False,
):
    xbufs = unroll * b_sh if loop > 0 else 2
    with tile.TileContext(nc) as tc:
        with (
            tc.tile_pool(name="wpool", bufs=1) as wpool,
            tc.tile_pool(name="xpool", bufs=xbufs) as xpool,
            tc.tile_pool(name="stage", bufs=6) as stage,
            tc.tile_pool(name="psum", bufs=8, space="PSUM") as pspool,
        ):
            wt = wpool.tile([128, 9, C_OUT], w_dt)
            bt = wpool.tile([C_OUT, 1], F32)
            nc.sync.dma_start(wt[:], w_d[:])
            nc.sync.dma_start(bt[:], b_d[:])

            def issue_loads(b):
                xt = xpool.tile([128, xcols], mm_dt, tag="x")
                for a, c in zip(bounds[:-1], bounds[1:]):
                    nc.sync.dma_start(xt[:, a:c], x_d[b, :, a:c])
                return xt

            def emit_warm():
                # junk matmuls on the resident weight tile keep the PE
                # HAM-warm across the head x-load window; the first
                # real start=True matmul clears the bank anyway.
                wv = wt[:].rearrange("p a b -> p (a b)")  # [128, 1152]
                pw = pspool.tile([C_OUT, nfree], F32, tag="ps")
                for j in range(4):
                    c0 = 456 * (j % 2)
                    nc.tensor.matmul(
                        pw[:, :],
                        wt[:, 0, :],
                        wv[:, c0 : c0 + nfree],
                        start=(j == 0),
                        stop=(j == 3),
                        skip_group_check=True,
                    )

            def compute(b, xt):
                if mmonly:
                    # structurally identical MM stream reading the resident
                    # weight tile instead of x; no loads, no evac/stores
                    wv = wt[:].rearrange("p a b -> p (a b)")

                def rhs(c0, lo=0, hi=128):
                    if mmonly:
                        cm = c0 % 696
                        return wv[lo:hi, cm : cm + nfree]
                    return xt[lo:hi, c0 : c0 + nfree]

                if True:
                    # batches of pair-units (2*bs PSUM banks), cfg-outer so
                    # each stationary operand streams bs rhs tiles in a row
                    for bi, ts in enumerate(batches):
                        pse, pso = {}, {}
                        for t in ts:
                            ps_te = pspool.tile([C_OUT, nfree], F32, tag="ps")
                            pse[t] = ps_te
                            ps_to = pspool.tile([C_OUT, nfree], F32, tag="ps")
                            pso[t] = ps_to
                        # even groups (out rows 8t, 8t+2, 8t+4, 8t+6)
                        for dx in range(3):
                            for t in ts:
                                c0 = (4 * t + 1) * WP + dx
                                nc.tensor.matmul(
                                    pse[t][:, :],
                                    wt[:, dx, :],
                                    rhs(c0),
                                    start=(dx == 0),
                                    stop=False,
                                )
                            if pe2x:  # timing diagnostic: 1.5x PE columns
                                for t in ts:
                                    c0 = (4 * t + 1) * WP + dx
                                    nc.tensor.matmul(
                                        pse[t][:, :],
                                        wt[:, dx, :],
                                        rhs(c0),
                                        start=False,
                                        stop=False,
                                        skip_group_check=True,
                                    )
                        # odd groups (out rows 8t+1 .. 8t+7)
                        for dx in range(3):
                            for t in ts:
                                c0 = (4 * t + 1) * WP + dx
                                nc.tensor.matmul(
                                    pso[t][:, :],
                                    wt[:, 3 + dx, :],
                                    rhs(c0),
                                    start=(dx == 0),
                                    stop=False,
                                )
                            if pe2x:
                                for t in ts:
                                    c0 = (4 * t + 1) * WP + dx
                                    nc.tensor.matmul(
                                        pso[t][:, :],
                                        wt[:, 3 + dx, :],
                                        rhs(c0),
                                        start=False,
                                        stop=False,
                                        skip_group_check=True,
                                    )
                        # K=64 leftovers on disjoint PE row halves: the
                        # even-group tap reads partitions 64:128, the
                        # odd-group tap partitions 0:64.
                        def emit_u(t, dx):
                            ce = (4 * t) * WP + dx
                            nc.tensor.matmul(
                                pse[t][:, :],
                                wt[64:128, 6 + dx, :],
                                rhs(ce, 64, 128),
                                start=False,
                                stop=(dx == 2),
                            )

                        def emit_l(t, dx):
                            co = (4 * t + 2) * WP + dx
                            nc.tensor.matmul(
                                pso[t][:, :],
                                wt[0:64, 6 + dx, :],
                                rhs(co, 0, 64),
                                start=False,
                                stop=(dx == 2),
                            )

                        if interleave:
                            # u/l alternate -> adjacent matmuls hit disjoint
                            # row groups and co-execute on the PE
                            for dx in range(3):
                                for t in ts:
                                    emit_u(t, dx)
                                    emit_l(t, dx)
                        else:
                            for dx in range(3):
                                for t in ts:
                                    emit_u(t, dx)
                                for t in ts:
                                    emit_l(t, dx)
                        # bias + evacuate PSUM -> staging [co, 4, 2, W]
                        if mmonly:
                            continue
                        for t in ts:
                            st = stage.tile([C_OUT, 4, 2, W], out_dt, tag="st")
                            ev = pse[t][:].rearrange("p (g w) -> p g w", w=WP)[
                                :, :, 0:W
                            ]
                            ov = pso[t][:].rearrange("p (g w) -> p g w", w=WP)[
                                :, :, 0:W
                            ]
                            nc.scalar.activation(
                                st[:, :, 0, :],
                                ev,
                                mybir.ActivationFunctionType.Identity,
                                bias=bt[:, 0:1],
                            )
                            nc.vector.tensor_scalar_add(
                                st[:, :, 1, :], ov, bt[:, 0:1]
                            )
                            if store_eng == "alt":
                                eng = nc.scalar if (t % 2 == 0) else nc.gpsimd
                            else:
                                eng = getattr(nc, store_eng)
                            eng.dma_start(o_d[b, :, 8 * t : 8 * t + 8, :], st[:])

            def emit_conv(warm: bool = False):
                if warm:
                    emit_warm()
                xts = [issue_loads(b) for b in range(b_sh)] if prefetch else None
                for b in range(b_sh):
                    xt = xts[b] if prefetch else issue_loads(b)
                    compute(b, xt)

            if loop > 0:
                # software-pipelined double buffering: each body runs
                # `unroll` convs; x for conv k+1 is DMA'd while conv k
                # computes, so the first matmul after a loop back-edge
                # never waits on HBM.  The last loads land in the same
                # pool bufs the next iteration's first computes read.
                assert loop % unroll == 0, "loop must divide by unroll"
                pre = (
                    [issue_loads(b) for b in range(b_sh)]
                    if not mmonly else [None] * b_sh
                )
                with tc.For_i(
                    0,
                    loop // unroll,
                    1,
                    hint_engines=(mybir.EngineType.PE,),
                    staggered_reset=staggered,
                ):
                    if keep_warm:
                        emit_warm()
                    cur = pre
                    for _u in range(unroll):
                        nxt = (
                            [issue_loads(b) for b in range(b_sh)]
                            if not mmonly else [None] * b_sh
                        )
                        for b in range(b_sh):
                            compute(b, cur[b])
                        cur = nxt
            else:
                for _rep in range(repeat):
                    emit_conv(warm=keep_warm)


class Runner:
    """Persistent jitted shard_map executor for a compiled Bass program.

    Mirrors concourse.bass2jax.run_bass_via_pjrt's multi-core path but
    caches the jitted function so repeated calls skip re-tracing.
    """

    def __init__(self, nc, n_cores: int = N_CORES):
        bass2jax.install_neuronx_cc_hook()
        assert nc.dbg_addr is None
        self.nc = nc
        self.n_cores = n_cores
        partition_name = (
            nc.partition_id_tensor.name if nc.partition_id_tensor else None
        )
        in_names: list[str] = []
        out_names: list[str] = []
        out_avals: list[jax.core.ShapedArray] = []
        for alloc in nc.m.functions[0].allocations:
            if not isinstance(alloc, mybir.MemoryLocationSet):
                continue
            name = alloc.memorylocations[0].name
            if alloc.kind == "ExternalInput":
                if name != partition_name:
                    in_names.append(name)
            elif alloc.kind == "ExternalOutput":
                out_names.append(name)
                out_avals.append(
                    jax.core.ShapedArray(
                        tuple(alloc.tensor_shape), mybir.dt.np(alloc.dtype)
                    )
                )
        self.in_names = in_names
        self.out_names = out_names
        self.out_avals = out_avals
        self.in_dtypes = {}
        for alloc in nc.m.functions[0].allocations:
            if (
                isinstance(alloc, mybir.MemoryLocationSet)
                and alloc.kind == "ExternalInput"
            ):
                self.in_dtypes[alloc.memorylocations[0].name] = mybir.dt.np(
                    alloc.dtype
                )
        n_params = len(in_names)
        n_outs = len(out_names)
        all_names = list(in_names) + list(out_names)
        if partition_name is not None:
            all_names.append(partition_name)
        all_names = tuple(all_names)

        def _body(*args):
            operands = list(args)
            if partition_name is not None:
                operands.append(bass2jax.partition_id_tensor())
            outs = bass2jax._bass_exec_p.bind(
                *operands,
                out_avals=tuple(out_avals),
                in_names=all_names,
                out_names=tuple(out_names),
                lowering_input_output_aliases=(),
                sim_require_finite=True,
                sim_require_nnan=True,
                nc=nc,
            )
            return tuple(outs)

        devices = jax.devices()[:n_cores]
        assert len(devices) == n_cores
        self.mesh = Mesh(np.asarray(devices), ("core",))
        in_specs = (PartitionSpec("core"),) * (n_params + n_outs)
        out_specs = (PartitionSpec("core"),) * n_outs
        donate = tuple(range(n_params, n_params + n_outs))
        self.fn = jax.jit(
            shard_map(
                _body,
                mesh=self.mesh,
                in_specs=in_specs,
                out_specs=out_specs,
                check_rep=False,
            ),
            donate_argnums=donate,
            keep_unused=True,
        )

    def concat_inputs(self, in_maps):
        return [
            np.concatenate(
                [
                    np.asarray(m[name]).astype(self.in_dtypes[name], copy=False)
                    for m in in_maps
                ],
                axis=0,
            )
            for name in self.in_names
        ]

    def zero_outs(self):
        return [
            np.zeros((self.n_cores * a.shape[0], *a.shape[1:]), a.dtype)
            for a in self.out_avals
        ]

    def call_raw(self, concat_in, zeros):
        """concat_in/zeros may be np or device arrays. Returns jax arrays."""
        return self.fn(*concat_in, *zeros)

    def __call__(self, in_maps):
        outs = self.call_raw(self.concat_inputs(in_maps), self.zero_outs())
        outs = [np.asarray(o) for o in outs]
        return [
            {
                name: outs[i].reshape(self.n_cores, *self.out_avals[i].shape)[c]
                for i, name in enumerate(self.out_names)
            }
            for c in range(self.n_cores)
        ]


_CACHE: dict = {}


def get_runner(repeat: int = 1, loop: int = 0, **kw) -> Runner:
    key = ("full", repeat, loop, tuple(sorted(kw.items())))
    if key not in _CACHE:
        nc = build(B // N_CORES, H, repeat=repeat, loop=loop, **kw)
        _CACHE[key] = Runner(nc)
    return _CACHE[key]


def make_in_maps(x, w, b):
    b_sh = B // N_CORES
    wp = _prep_w(np.asarray(w))
    bp = np.asarray(b).astype(np.float32).reshape(C_OUT, 1)
    xp = _prep_x(np.asarray(x, dtype=np.float32), H)
    return [
        {"xprep": xp[i * b_sh : (i + 1) * b_sh], "wprep": wp, "bias": bp}
        for i in range(N_CORES)
    ]


def kernel(x, w, b):
    runner = get_runner()
    res = runner(make_in_maps(x, w, b))
    out = np.concatenate([r["out"] for r in res], axis=0)
    return np.asarray(out, dtype=np.float32)
